# revision 1
# baseline (speedup 1.0000x reference)
import numpy as np

# Problem shapes (hardcoded from spec): x [131072,3]; per-cartesian-AO params:
# centers_ao [240,3], ls [240,3] int32, anorms [240], coeffs/zetas [240,6],
# normalization [240], cart2sph [240,224]. Output [131072,224] float32.
# Sharding: pure data parallel over the N=131072 point axis across 8 cores;
# all basis parameters are tiny and replicated.

N_CORES = 8


def _np_compute(x, centers_ao, ls, anorms, coeffs, zetas, normalization, cart2sph):
    # CPU fallback, chunked over points to bound memory.
    N = x.shape[0]
    S = cart2sph.shape[1]
    out = np.empty((N, S), dtype=np.float32)
    w = (anorms * normalization).astype(np.float32)  # [A]
    step = 8192
    for i in range(0, N, step):
        xb = x[i:i + step]                                    # [n,3]
        dx = xb[:, None, :] - centers_ao[None, :, :]          # [n,A,3]
        r2 = np.sum(dx * dx, axis=-1)                         # [n,A]
        # ls entries are in {0,1,2}: compute angular part branch-free.
        ang = np.ones(r2.shape, dtype=np.float32)
        for k in range(3):
            d = dx[..., k]
            l = ls[None, :, k]
            ang = ang * np.where(l == 0, 1.0, np.where(l == 1, d, d * d)).astype(np.float32)
        rad = np.sum(coeffs[None] * np.exp(-zetas[None] * r2[..., None]), axis=-1)
        phi = (w[None] * ang * rad).astype(np.float32)        # [n,A]
        out[i:i + step] = phi @ cart2sph
    return out


def kernel(**inputs):
    x = np.asarray(inputs["x"], dtype=np.float32)
    centers_ao = np.asarray(inputs["centers_ao"], dtype=np.float32)
    ls = np.asarray(inputs["ls"], dtype=np.int32)
    anorms = np.asarray(inputs["anorms"], dtype=np.float32)
    coeffs = np.asarray(inputs["coeffs"], dtype=np.float32)
    zetas = np.asarray(inputs["zetas"], dtype=np.float32)
    normalization = np.asarray(inputs["normalization"], dtype=np.float32)
    cart2sph = np.asarray(inputs["cart2sph"], dtype=np.float32)

    try:
        import jax
        import jax.numpy as jnp

        devs = jax.devices()
        nd = min(N_CORES, len(devs))
        N = x.shape[0]
        if N % nd != 0:
            raise RuntimeError("uneven shard")
        ls_f = ls.astype(np.float32)

        def compute(xs, centers_ao, ls_f, w, coeffs, zetas, cart2sph):
            dx = xs[:, None, :] - centers_ao[None, :, :]       # [n,A,3]
            r2 = jnp.sum(dx * dx, axis=-1)                     # [n,A]
            ang = jnp.ones_like(r2)
            for k in range(3):
                d = dx[..., k]
                l = ls_f[None, :, k]
                ang = ang * jnp.where(l == 0.0, 1.0, jnp.where(l == 1.0, d, d * d))
            rad = jnp.sum(coeffs[None] * jnp.exp(-zetas[None] * r2[..., None]), axis=-1)
            phi = w[None] * ang * rad
            return phi @ cart2sph

        pc = jax.pmap(compute, in_axes=(0, None, None, None, None, None, None),
                      devices=devs[:nd])
        xs = x.reshape(nd, N // nd, 3)
        w = (anorms * normalization).astype(np.float32)
        out = pc(xs, centers_ao, ls_f, w, coeffs, zetas, cart2sph)
        return np.asarray(out).reshape(N, cart2sph.shape[1]).astype(np.float32)
    except Exception:
        return _np_compute(x, centers_ao, ls, anorms, coeffs, zetas,
                           normalization, cart2sph)

